# revision 1
# baseline (speedup 1.0000x reference)
"""Trainium2 Bass kernel for nn_CompressedSensingConvolutional.

Problem: 200 FISTA iterations of
    re    = conv_transpose(x - conv(y_tmp, w_conv, stride 8, SAME) - b_conv,
                           w_ct, stride 8, SAME) + b_ct
    w     = y_tmp - re
    y_new = soft_threshold(w, lam)        (per-sample lam)
    y_tmp = y_new + beta_n (y_new - y_last)
with x: (64,9,9,3), output y_new: (64,72,72,3).

Structure exploited (all exact, no approximations):
  * conv_transpose with 5x5 kernel / stride 8 writes NON-overlapping 5x5
    patches at output rows/cols 8I+a, a in 0..4. Positions with row%8>4 or
    col%8>4 never receive an update, so (given b_ct=0 there) they stay 0
    bitwise forever. The live state is a 45x45x3 = [75=(a,b,ci), 9x9 blocks]
    phase-space grid of 6075 values per sample.
  * With y==0, w = -c where c = At(x - b_conv) + b_ct. If |c| <= lam
    elementwise, soft_threshold returns exactly 0 and the state never
    leaves 0: the sample's output is exactly zero. Only samples with
    max|c| > lam ("active") need the 200-iteration loop at all.
  * conv(y) restricted to the live grid is a 5x5 conv over the 9x9 block
    grid with 75 input channels -> 3 outputs = 25 shift-matmuls (K=75, M=3,
    N=81) accumulated in PSUM.
  * FISTA momentum commutes with the linear conv: A(y_new + b(y_new-y_last))
    is formed in the tiny 3x81 z-space from per-iteration conv outputs,
    so y_tmp is never materialized for the conv input.

Each active sample runs on its own NeuronCore (8 cores; extra actives are
handled in additional device rounds). All 200 iterations are unrolled with
state resident in SBUF/PSUM; no DMA inside the loop.
"""

import math
import os
import sys

import numpy as np

for _p in ("/opt/trn_rl_repo", "/root/.axon_site/_ro/trn_rl_repo"):
    if os.path.isdir(_p) and _p not in sys.path:
        sys.path.insert(0, _p)

N_ITERS = 200
N_CORES = 8
HW = 72
LOW = 9
C = 3
F = 75          # (a,b,ci): 5*5*3 live phase-space channels
NP2 = 13        # padded block grid (9 + 2 on each side)
NB = 9          # block grid
NPOS = NB * NB  # 81


def _betas(n_iters):
    """beta_n = (t_n - 1)/t_{n+1}, bit-exact fp32 mirror of the reference."""
    one, two, four = np.float32(1.0), np.float32(2.0), np.float32(4.0)
    t = np.float32(1.0)
    out = []
    for _ in range(n_iters):
        t_n = (one + np.sqrt(one + four * t * t)) / two
        out.append(float((t - one) / t_n))
        t = t_n
    return out


_DEV_CACHE = {}


def _build_device(n_iters):
    """Build + compile the per-core FISTA program (SPMD, same code all cores).

    Layout: one active sample per core. State y_tmp lives in a padded
    [75, 13x13] fp32 SBUF tile. Per iteration:
      z-phase : 25 shift-matmuls (5x5 phase conv, K=75, M=3, N=81) spread
                over 4 PSUM column groups (tile_position col packing) so 4
                streams overlap on the PE array; +1 matmul adds bx.
      reduce  : group partials summed into zca (SBUF) - 2 copies (1 on ACT)
                + 2 adds on DVE.
      At-phase: w = Wr^T zca + I75^T y_tmp accumulated in PSUM (2 matmuls);
                row 3 of zca is constant 1.0 so Wr row 3 folds in -b_ct.
      soft    : cl = clamp(w, +-lam); y_new = w - cl   (DVE, reads PSUM)
      momentum: y_tmp' = (1+beta_n) y_new - beta_n y_last, with
                e = beta_n*y_last issued early (overlaps the z-phase).
    """
    if n_iters in _DEV_CACHE:
        return _DEV_CACHE[n_iters]

    import concourse.bacc as bacc
    import concourse.mybir as mybir
    from concourse.tile import TileContext

    f32 = mybir.dt.float32
    Alu = mybir.AluOpType

    betas = [float(b) for b in _betas(max(n_iters, 1))]

    nc = bacc.Bacc(trn_type="TRN2")
    wc_d = nc.dram_tensor("wc", [F, 75], f32, kind="ExternalInput")
    wcomb_d = nc.dram_tensor("wcomb", [100, F], f32, kind="ExternalInput")
    bx_d = nc.dram_tensor("bx", [C, NPOS], f32, kind="ExternalInput")
    i3_d = nc.dram_tensor("i3", [C, C], f32, kind="ExternalInput")
    lam_d = nc.dram_tensor("lam2", [F, 2], f32, kind="ExternalInput")
    y_d = nc.dram_tensor("y", [F, NPOS], f32, kind="ExternalOutput")

    # shift s -> column group; g0 gets 7 shifts, g1 6 (+bx), g2/g3 6.
    grp_of = [s % 4 for s in range(25)]
    order = []  # round-robin issue order for concurrency
    by_g = [[s for s in range(25) if grp_of[s] == g] for g in range(4)]
    for r in range(7):
        for g in range(4):
            if r < len(by_g[g]):
                order.append(by_g[g][r])

    with TileContext(nc) as tc:
        with tc.tile_pool(name="const", bufs=1) as cpool, \
             tc.tile_pool(name="state", bufs=1) as spool, \
             tc.tile_pool(name="work", bufs=3) as wpool, \
             tc.tile_pool(name="psum", bufs=2, space="PSUM") as ppool:
            wc = cpool.tile([F, 75], f32, tag="wc")
            nc.sync.dma_start(wc[:], wc_d[:])
            wcomb = cpool.tile([100, F], f32, tag="wcomb")
            nc.sync.dma_start(wcomb[:], wcomb_d[:])
            bx = cpool.tile([C, NPOS], f32, tag="bx")
            nc.sync.dma_start(bx[:], bx_d[:])
            i3 = cpool.tile([C, C], f32, tag="i3")
            nc.sync.dma_start(i3[:], i3_d[:])
            lam2 = cpool.tile([F, 2], f32, tag="lam")
            nc.sync.dma_start(lam2[:], lam_d[:])

            # zy rows 0:75 = y_tmp (padded); rows 96:99 = zcomb; row 99 = ones.
            # One tile so the At-phase is a single K=100 matmul.
            zy = spool.tile([100, NP2 * NP2], f32, tag="zy")
            yn = [spool.tile([F, NPOS], f32, tag=f"yn{i}", name=f"yn{i}")
                  for i in range(2)]
            nc.vector.memset(zy[:], 0.0)
            nc.vector.memset(zy[96:100, :], 1.0)  # row 99 stays 1.0
            nc.vector.memset(yn[0][:], 0.0)
            nc.vector.memset(yn[1][:], 0.0)

            zyv = zy[:].rearrange("p (r c) -> p r c", c=NP2)
            ytv = zyv[0:F]
            yt_int = ytv[:, 2:2 + NB, 2:2 + NB]
            zc_int = zyv[96:99, 2:2 + NB, 2:2 + NB]
            comb_int = zyv[:, 2:2 + NB, 2:2 + NB]

            for n in range(n_iters):
                beta = betas[n]
                ynew = yn[n % 2]
                ylast = yn[(n + 1) % 2]

                # early: e = beta_n * y_last (overlaps z-phase)
                e = wpool.tile([F, NPOS], f32, tag="e")
                nc.vector.tensor_scalar_mul(e[:], ylast[:], beta)

                # z-phase: a~ = A_lin(y_tmp) + bx in 4 PSUM column groups
                pz = ppool.tile([128, NPOS], f32, tag="pz")
                nc.tensor.matmul(pz[32:35, :], i3[:], bx[:], start=True, stop=False,
                                 tile_position=(0, 32))
                seen = [0, 0, 0, 0]
                for s in order:
                    g = grp_of[s]
                    m, nn_ = divmod(s, 5)
                    nc.tensor.matmul(
                        pz[32 * g:32 * g + 3, :],
                        wc[:, 3 * s:3 * s + 3],
                        ytv[:, m:m + NB, nn_:nn_ + NB],
                        start=(seen[g] == 0 and g != 1),
                        stop=(seen[g] == len(by_g[g]) - 1),
                        tile_position=(0, 32 * g),
                    )
                    seen[g] += 1

                # reduce groups: zca[0:3] = (P0+P1) + (P2+P3)
                h1 = wpool.tile([3, NPOS], f32, tag="h1")
                nc.scalar.copy(h1[:], pz[32:35, :])
                h2 = wpool.tile([3, NPOS], f32, tag="h2")
                nc.vector.tensor_copy(h2[:], pz[96:99, :])
                s1 = wpool.tile([3, NPOS], f32, tag="s1")
                nc.vector.tensor_add(s1[:], pz[0:3, :], h1[:])
                s2 = wpool.tile([3, NPOS], f32, tag="s2")
                nc.vector.tensor_add(s2[:], pz[64:67, :], h2[:])
                nc.vector.tensor_add(zc_int, s1[:], s2[:])

                # At-phase: w = Wr^T zcomb + y_tmp as ONE K=100 matmul
                pw = ppool.tile([F, NPOS], f32, tag="pw")
                nc.tensor.matmul(pw[:], wcomb[:], comb_int, start=True, stop=True)

                # soft threshold (reads PSUM)
                cl = wpool.tile([F, NPOS], f32, tag="cl")
                nc.vector.tensor_scalar(
                    cl[:], pw[:], lam2[:, 0:1], lam2[:, 1:2], Alu.min, Alu.max
                )
                nc.vector.tensor_sub(ynew[:], pw[:], cl[:])

                # momentum: y_tmp' = (1+beta)*y_new - e
                f = wpool.tile([F, NPOS], f32, tag="f")
                nc.vector.tensor_scalar_mul(f[:], ynew[:], 1.0 + beta)
                nc.vector.tensor_sub(yt_int, f[:], e[:])

            nc.sync.dma_start(y_d[:], yn[(n_iters - 1) % 2][:])

    nc.compile()
    _DEV_CACHE[n_iters] = nc
    return nc


def kernel(x, lam, w_conv, b_conv, w_ct, b_ct):
    from concourse import bass_utils

    x = np.asarray(x, np.float32)
    lam = np.asarray(lam, np.float32)
    w_conv = np.asarray(w_conv, np.float32)
    b_conv = np.asarray(b_conv, np.float32)
    w_ct = np.asarray(w_ct, np.float32)
    b_ct = np.asarray(b_ct, np.float32)
    B = x.shape[0]

    # ---- host analysis (exact): c = At(x - b_conv) + b_ct on the live grid
    w_rev = w_ct[::-1, ::-1]                      # [a,b,ci,co] = w_ct[4-a,4-b,ci,co]
    xb = x - b_conv                               # (B,9,9,3)
    # c[s, a, b, co, I, J]
    c = np.einsum('abeo,sije->sabo' 'ij', w_rev, xb, optimize=True)
    c = c + b_ct[None, None, None, :, None, None]
    cmax = np.abs(c).max(axis=(1, 2, 3, 4, 5))
    active = cmax > lam * np.float32(1.0 - 1e-5)
    act_idx = np.where(active)[0]

    # ---- device weights (same for every core)
    aa, bb_, cc = np.meshgrid(np.arange(5), np.arange(5), np.arange(C), indexing='ij')
    # Wc_all[f=(a,b,ci), 3*s+co] = w_conv[8m+a, 8n+b, ci, co],  s = 5m+n
    Wc_all = np.zeros((F, 75), np.float32)
    for s in range(25):
        m, n = divmod(s, 5)
        blk = w_conv[8 * m + aa, 8 * n + bb_, cc, :]      # (5,5,3,3)
        Wc_all[:, 3 * s:3 * s + 3] = blk.reshape(F, C)
    # Wcomb: rows 0:75 identity (adds y_tmp); rows 96:99 = Wr (At weights,
    # [ci, (a,b,co)] = w_rev[a,b,ci,co]); row 99 = -b_ct (ones row in zy).
    Wcomb = np.zeros((100, F), np.float32)
    Wcomb[0:F, :] = np.eye(F, dtype=np.float32)
    Wcomb[96:99, :] = np.transpose(w_rev, (2, 0, 1, 3)).reshape(C, F)
    Wcomb[99, :] = np.broadcast_to(-b_ct, (5, 5, C)).reshape(F)
    I3 = np.eye(C, dtype=np.float32)

    out = np.zeros((B, HW, HW, C), np.float32)

    # Non-patch positions evolve autonomously: w = y - b_ct per channel.
    # Exact when b_ct == 0 (it is, per the model); otherwise computed here.
    if np.any(b_ct != 0.0):
        betas = _betas(N_ITERS)
        yv = np.zeros((B, C), np.float32)
        yl = np.zeros((B, C), np.float32)
        for n in range(N_ITERS):
            w_np = yv - b_ct[None, :]
            y_new = (np.maximum(w_np - lam[:, None], 0)
                     - np.maximum(-w_np - lam[:, None], 0)).astype(np.float32)
            yv = y_new + np.float32(betas[n]) * (y_new - yl)
            yl = y_new
        mask = np.ones((HW, HW), bool)
        rows = (np.arange(HW) % 8) < 5
        mask[np.ix_(rows, rows)] = False          # live-grid positions
        out[:, mask, :] = yl[:, None, :]

    nc = _build_device(N_ITERS)

    n_rounds = max(1, math.ceil(len(act_idx) / N_CORES))
    zero_bx = np.zeros((C, NPOS), np.float32)
    one_lam = np.stack([np.ones(F, np.float32), -np.ones(F, np.float32)], axis=1)
    for r in range(n_rounds):
        batch = act_idx[r * N_CORES:(r + 1) * N_CORES]
        in_maps = []
        for k in range(N_CORES):
            if k < len(batch):
                s = int(batch[k])
                bx = np.ascontiguousarray(
                    (b_conv[:, None] - x[s].reshape(NPOS, C).T).astype(np.float32))
                lam2 = np.stack([np.full(F, lam[s], np.float32),
                                 np.full(F, -lam[s], np.float32)], axis=1)
            else:
                bx, lam2 = zero_bx, one_lam
            in_maps.append({
                "wc": Wc_all, "wcomb": Wcomb, "bx": bx, "i3": I3,
                "lam2": np.ascontiguousarray(lam2),
            })
        res = bass_utils.run_bass_kernel_spmd(nc, in_maps, core_ids=list(range(N_CORES)))
        for k in range(len(batch)):
            s = int(batch[k])
            ya = res.results[k]["y"].reshape(5, 5, C, NB, NB)
            # out[s, 8I+a, 8J+b, ci] = ya[a,b,ci,I,J]
            blk = np.transpose(ya, (3, 0, 4, 1, 2))   # (I,a,J,b,ci)
            ov = out[s].reshape(NB, 8, NB, 8, C)
            ov[:, :5, :, :5, :] = blk
    return out



# revision 2
# speedup vs baseline: 47.1901x; 47.1901x over previous
"""Trainium2 Bass kernel for nn_CompressedSensingConvolutional (hw-loop version).

Problem: 200 FISTA iterations of
    re    = conv_transpose(x - conv(y_tmp, w_conv, stride 8, SAME) - b_conv,
                           w_ct, stride 8, SAME) + b_ct
    w     = y_tmp - re
    y_new = soft_threshold(w, lam)        (per-sample lam)
    y_tmp = y_new + beta_n (y_new - y_last)
with x: (64,9,9,3), output y_new: (64,72,72,3).

Structure exploited (all exact, no approximations):
  * conv_transpose with 5x5 kernel / stride 8 writes NON-overlapping 5x5
    patches; the live state is 45x45x3 = [75=(a,b,co), 9x9 blocks] per
    sample.  Samples whose max|At(x)| <= lam stay exactly 0 and skip the
    device entirely.
  * conv(y) on the live grid is a 5x5 conv over the 9x9 block grid with
    75 input channels -> 3 outputs = 25 shift-matmuls (K=75, M=3, N=81)
    accumulated in PSUM across 4 tile_position column groups.
  * Momentum commutes with the conv: A(y_tmp_{n+1}) = s_n A(y_new_n)
    - b_n A(y_new_{n-1}), so the conv runs on y_new right after
    thresholding and the scaled images live in two ping-pong SBUF tiles
    (zmov) whose 4 group-slots sit at the 32-aligned partition starts the
    engines require.  The prev tile is rescaled in place by
    r_n = -beta_n/s_{n-1} each iteration.
  * The At-phase w = y_tmp + Wr(A(y_tmp) + bx) - b_ct is THREE accumulating
    matmuls with CONSTANT stationaries: [I75|Wr(bx)|-b_ct] @ ytmp-tile,
    Wr-pattern @ zmov_cur, Wr-pattern @ zmov_prev.
  * y_tmp_{n+1} = s_n y_new_n - b_n y_new_{n-1} is materialized on GpSimd,
    fully overlapped with the conv matmuls.

All 200 iterations run inside ONE tc.For_i hardware loop (2 iterations
per trip, ping-pong buffers); per-iteration scalars (s_n = 1+beta_n,
b_n = beta_n, r_n = -beta_n/s_{n-1}) come from a small SBUF table via
dynamically sliced [P,1] scalar APs.  Program is ~110 instructions vs
~7200 fully unrolled, which slashes both NEFF build/load overhead and
device time.

Each active sample runs on its own NeuronCore (8 cores; extra actives are
handled in additional device rounds).
"""

import math
import os
import sys

import numpy as np

for _p in ("/opt/trn_rl_repo", "/root/.axon_site/_ro/trn_rl_repo"):
    if os.path.isdir(_p) and _p not in sys.path:
        sys.path.insert(0, _p)

N_ITERS = 200
N_CORES = 8
HW = 72
LOW = 9
C = 3
F = 75          # (a,b,co): 5*5*3 live phase-space channels
NP2 = 13        # padded block grid (9 + 2 on each side)
NB = 9          # block grid
NPOS = NB * NB  # 81
SB = 100        # sched table block stride (max 100 trips per parity)


def _betas(n_iters):
    """beta_n = (t_n - 1)/t_{n+1}, bit-exact fp32 mirror of the reference."""
    one, two, four = np.float32(1.0), np.float32(2.0), np.float32(4.0)
    t = np.float32(1.0)
    out = []
    for _ in range(n_iters):
        t_n = (one + np.sqrt(one + four * t * t)) / two
        out.append(np.float32((t - one) / t_n))
        t = t_n
    return out


_DEV_CACHE = {}


def _build_device(n_iters):
    """Build + compile the per-core FISTA program (SPMD, same code all cores)."""
    if n_iters in _DEV_CACHE:
        return _DEV_CACHE[n_iters]
    assert n_iters <= 2 * SB

    import concourse.bacc as bacc
    import concourse.mybir as mybir
    from concourse.bass import ds
    from concourse.tile import TileContext

    f32 = mybir.dt.float32
    Alu = mybir.AluOpType

    # shift s -> column group; g0 gets 7 shifts, g1/g2/g3 6.
    grp_of = [s % 4 for s in range(25)]
    by_g = [[s for s in range(25) if grp_of[s] == g] for g in range(4)]
    order = []  # round-robin issue order for concurrency
    for rr in range(7):
        for g in range(4):
            if rr < len(by_g[g]):
                order.append(by_g[g][rr])

    nc = bacc.Bacc(trn_type="TRN2")
    wc_d = nc.dram_tensor("wc", [F, 75], f32, kind="ExternalInput")
    wta_d = nc.dram_tensor("wta", [100, F], f32, kind="ExternalInput")
    wtb_d = nc.dram_tensor("wtb", [99, F], f32, kind="ExternalInput")
    bxo_d = nc.dram_tensor("bxo", [4, NPOS], f32, kind="ExternalInput")
    lam_d = nc.dram_tensor("lam2", [F, 2], f32, kind="ExternalInput")
    sched_d = nc.dram_tensor("sched", [128, 6 * SB], f32, kind="ExternalInput")
    y_d = nc.dram_tensor("y", [F, NPOS], f32, kind="ExternalOutput")

    with TileContext(nc) as tc:
        with tc.tile_pool(name="const", bufs=1) as cpool, \
             tc.tile_pool(name="state", bufs=1) as spool, \
             tc.tile_pool(name="work", bufs=3) as wpool, \
             tc.tile_pool(name="psum", bufs=2, space="PSUM") as ppool:
            wc = cpool.tile([F, 75], f32, tag="wc")
            nc.sync.dma_start(wc[:], wc_d[:])
            wta = cpool.tile([100, F], f32, tag="wta")
            nc.sync.dma_start(wta[:], wta_d[:])
            wtb = cpool.tile([99, F], f32, tag="wtb")
            nc.sync.dma_start(wtb[:], wtb_d[:])
            lam2 = cpool.tile([F, 2], f32, tag="lam")
            nc.sync.dma_start(lam2[:], lam_d[:])
            sched = cpool.tile([128, 6 * SB], f32, tag="sched")
            nc.sync.dma_start(sched[:], sched_d[:])

            # ytmp rows 0:75 = y_tmp; 96:99 = bx (static); 99 = ones.
            ytmp = spool.tile([100, NPOS], f32, tag="ytmp")
            nc.vector.memset(ytmp[:], 0.0)
            nc.sync.dma_start(ytmp[96:100, :], bxo_d[:])
            # zmov ping-pong: rows 32g..32g+2 = scaled conv image of group g
            zmov = [spool.tile([99, NPOS], f32, tag=f"zm{i}", name=f"zm{i}")
                    for i in range(2)]
            yn = [spool.tile([F, NP2 * NP2], f32, tag=f"yn{i}", name=f"yn{i}")
                  for i in range(2)]
            for t in zmov + yn:
                nc.vector.memset(t[:], 0.0)

            ynv = [t[:].rearrange("p (r c) -> p r c", c=NP2) for t in yn]
            yn_int = [v[:, 2:2 + NB, 2:2 + NB] for v in ynv]

            def emit_iter(idx, p, dyn):
                """One FISTA iteration. idx: trip index (ScalarValue if dyn
                else python int); p: parity (0 even, 1 odd)."""
                def scol(rows, block):
                    if dyn:
                        return sched[rows, ds(idx + block * SB, 1)]
                    col = block * SB + idx
                    return sched[rows, col:col + 1]

                # blocks: 0 s_e, 1 s_o, 2 b_e, 3 b_o, 4 r_e, 5 r_o
                blk_s, blk_b, blk_r = p, 2 + p, 4 + p
                cur, prev = zmov[1 - p], zmov[p]

                # At-phase: w_n = y_tmp + Wr(A(y_tmp) + bx) - b_ct
                pw = ppool.tile([F, NPOS], f32, tag="pw")
                nc.tensor.matmul(pw[:], wta[:], ytmp[:], start=True, stop=False)
                nc.tensor.matmul(pw[:], wtb[:], cur[:], start=False, stop=False)
                nc.tensor.matmul(pw[:], wtb[:], prev[:], start=False, stop=True)

                # rescale what was "cur" for use as "prev" next iteration
                nc.gpsimd.tensor_scalar_mul(cur[:], cur[:], scol(slice(0, 99), blk_r))

                # soft threshold: ynew = pw - clamp(pw, +-lam)
                cl = wpool.tile([F, NPOS], f32, tag="cl")
                nc.vector.tensor_scalar(
                    cl[:], pw[:], lam2[:, 0:1], lam2[:, 1:2], Alu.min, Alu.max)
                nc.vector.tensor_sub(yn_int[p], pw[:], cl[:])

                # conv on y_new: 25 shift-matmuls in 4 PSUM column groups
                pz = ppool.tile([128, NPOS], f32, tag="pz")
                seen = [0, 0, 0, 0]
                for s in order:
                    g = grp_of[s]
                    m, nn_ = divmod(s, 5)
                    nc.tensor.matmul(
                        pz[32 * g:32 * g + 3, :],
                        wc[:, 3 * s:3 * s + 3],
                        ynv[p][:, m:m + NB, nn_:nn_ + NB],
                        start=(seen[g] == 0),
                        stop=(seen[g] == len(by_g[g]) - 1),
                        tile_position=(0, 32 * g),
                    )
                    seen[g] += 1

                # scaled image copies into the new "cur" (= prev buffer)
                for g in range(4):
                    nc.vector.tensor_scalar_mul(
                        prev[32 * g:32 * g + 3, :], pz[32 * g:32 * g + 3, :],
                        scol(slice(0, 3), blk_s))

                # momentum (overlaps conv): y_tmp' = s_n*ynew - b_n*ylast
                u = wpool.tile([F, NPOS], f32, tag="u")
                nc.gpsimd.tensor_scalar_mul(
                    u[:], yn_int[1 - p], scol(slice(0, 75), blk_b))
                f = wpool.tile([F, NPOS], f32, tag="f")
                nc.gpsimd.tensor_scalar_mul(
                    f[:], yn_int[p], scol(slice(0, 75), blk_s))
                nc.gpsimd.tensor_sub(ytmp[0:F, :], f[:], u[:])

            trips = n_iters // 2
            tail = n_iters % 2
            if trips > 0:
                with tc.For_i(0, trips, 1) as i:
                    emit_iter(i, 0, True)
                    emit_iter(i, 1, True)
            if tail:
                emit_iter(trips, 0, False)

            last = (n_iters - 1) % 2 if n_iters > 0 else 1
            nc.sync.dma_start(y_d[:], yn_int[last])

    nc.compile()
    _DEV_CACHE[n_iters] = nc
    return nc


def _host_tables(w_conv, b_conv, w_ct, b_ct, n_iters=N_ITERS):
    """Device weight tables (sample-independent)."""
    w_rev = w_ct[::-1, ::-1]                      # [a,b,ci,co]
    aa, bb_, cc = np.meshgrid(np.arange(5), np.arange(5), np.arange(C),
                              indexing='ij')
    Wc_all = np.zeros((F, 75), np.float32)
    for s in range(25):
        m, n = divmod(s, 5)
        blk = w_conv[8 * m + aa, 8 * n + bb_, cc, :]      # (5,5,3,3)
        Wc_all[:, 3 * s:3 * s + 3] = blk.reshape(F, C)
    Wr = np.transpose(w_rev, (2, 0, 1, 3)).reshape(C, F)  # [ci, (a,b,co)]

    Wta = np.zeros((100, F), np.float32)
    Wta[0:F, :] = np.eye(F, dtype=np.float32)
    Wta[96:99, :] = Wr                  # bx rows
    Wta[99, :] = np.broadcast_to(-b_ct, (5, 5, C)).reshape(F)
    Wtb = np.zeros((99, F), np.float32)
    for g in range(4):
        Wtb[32 * g:32 * g + 3, :] = Wr

    betas = _betas(max(n_iters, 1))
    s_arr = np.array([np.float32(1.0) + b for b in betas], np.float32)
    b_arr = np.array(betas, np.float32)
    r_arr = np.zeros(len(betas), np.float32)
    for n in range(len(betas)):
        s_prev = s_arr[n - 1] if n > 0 else np.float32(1.0)
        r_arr[n] = np.float32(-(b_arr[n] / s_prev))

    sched = np.zeros((128, 6 * SB), np.float32)
    for n in range(n_iters):
        p, i = n % 2, n // 2
        sched[:, p * SB + i] = s_arr[n]            # blocks 0/1: s
        sched[:, (2 + p) * SB + i] = b_arr[n]      # blocks 2/3: b
        sched[:, (4 + p) * SB + i] = r_arr[n]      # blocks 4/5: r
    return Wc_all, Wta, Wtb, w_rev, sched


def kernel(x, lam, w_conv, b_conv, w_ct, b_ct):
    from concourse import bass_utils

    x = np.asarray(x, np.float32)
    lam = np.asarray(lam, np.float32)
    w_conv = np.asarray(w_conv, np.float32)
    b_conv = np.asarray(b_conv, np.float32)
    w_ct = np.asarray(w_ct, np.float32)
    b_ct = np.asarray(b_ct, np.float32)
    B = x.shape[0]

    Wc_all, Wta, Wtb, w_rev, sched = _host_tables(w_conv, b_conv, w_ct, b_ct)

    # ---- host analysis (exact): c = At(x - b_conv) + b_ct on the live grid
    xb = x - b_conv                               # (B,9,9,3)
    c = np.einsum('abeo,sije->sabo' 'ij', w_rev, xb, optimize=True)
    c = c + b_ct[None, None, None, :, None, None]
    cmax = np.abs(c).max(axis=(1, 2, 3, 4, 5))
    active = cmax > lam * np.float32(1.0 - 1e-5)
    act_idx = np.where(active)[0]

    out = np.zeros((B, HW, HW, C), np.float32)

    # Non-patch positions evolve autonomously: w = y - b_ct per channel.
    # Exact when b_ct == 0 (it is, per the model); otherwise computed here.
    if np.any(b_ct != 0.0):
        betas = _betas(N_ITERS)
        yv = np.zeros((B, C), np.float32)
        yl = np.zeros((B, C), np.float32)
        for n in range(N_ITERS):
            w_np = yv - b_ct[None, :]
            y_new = (np.maximum(w_np - lam[:, None], 0)
                     - np.maximum(-w_np - lam[:, None], 0)).astype(np.float32)
            yv = y_new + np.float32(betas[n]) * (y_new - yl)
            yl = y_new
        mask = np.ones((HW, HW), bool)
        rows = (np.arange(HW) % 8) < 5
        mask[np.ix_(rows, rows)] = False          # live-grid positions
        out[:, mask, :] = yl[:, None, :]

    nc = _build_device(N_ITERS)

    n_rounds = max(1, math.ceil(len(act_idx) / N_CORES))
    zero_bxo = np.zeros((4, NPOS), np.float32)
    zero_bxo[3, :] = 1.0
    one_lam = np.stack([np.ones(F, np.float32), -np.ones(F, np.float32)],
                       axis=1)
    for r in range(n_rounds):
        batch = act_idx[r * N_CORES:(r + 1) * N_CORES]
        in_maps = []
        for k in range(N_CORES):
            if k < len(batch):
                s = int(batch[k])
                bxo = np.empty((4, NPOS), np.float32)
                bxo[0:3] = b_conv[:, None] - x[s].reshape(NPOS, C).T
                bxo[3, :] = 1.0
                lam2 = np.stack([np.full(F, lam[s], np.float32),
                                 np.full(F, -lam[s], np.float32)], axis=1)
            else:
                bxo, lam2 = zero_bxo, one_lam
            in_maps.append({
                "wc": Wc_all, "wta": Wta, "wtb": Wtb,
                "bxo": np.ascontiguousarray(bxo),
                "lam2": np.ascontiguousarray(lam2), "sched": sched,
            })
        res = bass_utils.run_bass_kernel_spmd(nc, in_maps,
                                              core_ids=list(range(N_CORES)))
        for k in range(len(batch)):
            s = int(batch[k])
            ya = res.results[k]["y"].reshape(5, 5, C, NB, NB)
            # out[s, 8I+a, 8J+b, co] = ya[a,b,co,I,J]
            blk = np.transpose(ya, (3, 0, 4, 1, 2))   # (I,a,J,b,co)
            ov = out[s].reshape(NB, 8, NB, 8, C)
            ov[:, :5, :, :5, :] = blk
    return out
